# revision 13
# baseline (speedup 1.0000x reference)
"""AdaptiveFieldEvolution Trainium2 kernel (8 NeuronCores, SPMD).

Semantics (matches the fp32 reference): 50 Euler steps of
  f += DT*(dcoeff*depthwise3x3(f) + db + W2@gelu(W1@f + b1) + b2)
with a global freeze once mean|df| < THRESH, and an adaptive `steps` output.

Sharding: batch(4) x H-halves(2) -> 8 cores. Per-core tile [128, 4096]:
partition = slab(2)*64 + channel(64), free = y(32) x W(128); each slab is a
32-row band. fp32 master field, fp16 compute, fp32 PSUM accumulation.

Per step: PE runs the two 1x1-conv matmuls plus the six dx!=0 depthwise taps
(diagonal lhsT, PSUM-accumulated) and all boundary-row corrections; ACT runs
gelu (+b1, fp16 out) and the |deriv| abs-sum; DVE runs the three dx==0 taps
as an fp16 chain (+b2 tile) plus the PSUM merge and the fp32 master update;
GpSimd does the fp32->fp16 padded cast and the convergence bookkeeping.
One AllGather per step carries both halo edge rows + the local |deriv| sum;
per-core *data* (zeroed halo-tap weights at the global boundary, a partner
row index consumed via a register-offset DMA) keeps the SPMD graph identical
on all cores.
"""
import sys
import types
import numpy as np

sys.path.insert(0, "/opt/trn_rl_repo")


def _install_prof_shim():
    try:
        from antenv import axon_hooks  # noqa: F401
        return
    except ImportError:
        pass
    import antenv
    mod = types.ModuleType("antenv.axon_hooks")
    _state = {"hook": None}
    mod.set_axon_ntff_profile_hook = lambda h: _state.__setitem__("hook", h)
    mod.get_axon_ntff_profile_hook = lambda: _state["hook"]
    sys.modules["antenv.axon_hooks"] = mod
    antenv.axon_hooks = mod
    try:
        from trn_agent_boot.trn_boot import _ntff_profile_via_ctypes
        mod.set_axon_ntff_profile_hook(
            _ntff_profile_via_ctypes("/opt/axon/libaxon_pjrt.so"))
    except Exception:
        pass


_install_prof_shim()

import concourse.bass as bass  # noqa: E402
import concourse.tile as tile  # noqa: E402
from concourse import bacc, mybir, bass_isa  # noqa: E402
from concourse.bass_utils import run_bass_kernel_spmd  # noqa: E402
from contextlib import ExitStack  # noqa: E402

B, C, H, W = 4, 64, 128, 128
NCORES = 8
DT = 0.1
THRESH = 0.01
SLAB = 32                      # rows per slab half
FREE = SLAB * W                # 4096 real free elems per partition
XP = W + 2                     # padded row stride
FPAD = 2 + SLAB * XP + 2       # padded fp16 free size, front+tail pads
PAYLOAD = 2 * C * W            # top-row block + bottom-row block
NCHUNK = 4
CH = FREE // NCHUNK            # 1024 (one PSUM tile = 2 banks)
RPC = SLAB // NCHUNK           # 8 rows per chunk
F32d = mybir.dt.float32
F16d = mybir.dt.float16
THRESH_SUM = THRESH * (B * C * H * W) / DT

_NC_CACHE = {}


def build_nc(nsteps):
    nc = bacc.Bacc("TRN2", target_bir_lowering=False, debug=False,
                   num_devices=NCORES)
    f0_e = nc.dram_tensor("f0", [128, FREE], F32d, kind="ExternalInput")
    w1t_e = nc.dram_tensor("w1t", [128, 128], F16d, kind="ExternalInput")
    w2t_e = nc.dram_tensor("w2t", [128, 64], F16d, kind="ExternalInput")
    b1v_e = nc.dram_tensor("b1v", [128, 1], F32d, kind="ExternalInput")
    b2t_e = nc.dram_tensor("b2t", [128, FREE], F16d, kind="ExternalInput")
    stap_e = nc.dram_tensor("stap", [128, 9], F32d, kind="ExternalInput")
    dtap_e = nc.dram_tensor("dtap", [128, 9 * 128], F16d, kind="ExternalInput")
    d64_e = nc.dram_tensor("d64", [128, 3 * 64], F16d, kind="ExternalInput")
    dhalo_e = nc.dram_tensor("dhalo", [64, 6 * 64], F16d, kind="ExternalInput")
    hmask_e = nc.dram_tensor("hmask", [64, 4], F32d, kind="ExternalInput")
    outf_e = nc.dram_tensor("outf", [128, FREE], F32d, kind="ExternalOutput")
    steps_e = nc.dram_tensor("steps", [1, 1], mybir.dt.int32, kind="ExternalOutput")

    # dy=0 taps first: the per-bank opener must cover every row/col/partition
    TAPS_PE = [(0, -1), (0, 1), (-1, -1), (-1, 1), (1, -1), (1, 1)]

    with tile.TileContext(nc) as tc, ExitStack() as ctx:
        cpool = ctx.enter_context(tc.tile_pool(name="consts", bufs=1))
        spool = ctx.enter_context(tc.tile_pool(name="state", bufs=1))
        hpsum = ctx.enter_context(tc.tile_pool(name="hpsum", bufs=2, space="PSUM"))
        dpsum = ctx.enter_context(tc.tile_pool(name="dpsum", bufs=2, space="PSUM"))
        dram = ctx.enter_context(tc.tile_pool(name="dram", bufs=2, space="DRAM"))

        # ---- constants ----
        w1t = cpool.tile([128, 128], F16d, tag="w1t")
        w2t = cpool.tile([128, 64], F16d, tag="w2t")
        b1v = cpool.tile([128, 1], F32d, tag="b1v")
        b2t = cpool.tile([128, FREE], F16d, tag="b2t")
        stap = cpool.tile([128, 9], F32d, tag="stap")
        dtap = cpool.tile([128, 9 * 128], F16d, tag="dtap")
        d64 = cpool.tile([128, 3 * 64], F16d, tag="d64")
        dhalo = cpool.tile([64, 6 * 64], F16d, tag="dhalo")
        hmask = cpool.tile([64, 4], F32d, tag="hmask")
        for t, e in ((w1t, w1t_e), (w2t, w2t_e), (b1v, b1v_e), (b2t, b2t_e),
                     (stap, stap_e), (dtap, dtap_e), (d64, d64_e),
                     (dhalo, dhalo_e), (hmask, hmask_e)):
            nc.sync.dma_start(t[:], e[:])

        # ---- state ----
        fa = spool.tile([128, FREE], F32d, tag="fa")
        fb = spool.tile([128, FREE], F32d, tag="fb")
        f16p = spool.tile([128, FPAD], F16d, tag="f16p")
        h0 = spool.tile([128, FREE], F16d, tag="h0")
        h1 = spool.tile([128, FREE], F16d, tag="h1")
        hs = (h0, h1)
        acc = spool.tile([128, FREE], F16d, tag="acc")
        der = spool.tile([128, FREE], F16d, tag="der")
        junk = spool.tile([128, FREE], F16d, tag="junk")
        halo_t32 = spool.tile([64, W], F32d, tag="ht32")
        halo_b32 = spool.tile([64, W], F32d, tag="hb32")
        cand = [spool.tile([64, W], F32d, tag=f"cand{i}", name=f"cand{i}")
                for i in range(4)]
        halo_t16 = spool.tile([64, XP], F16d, tag="ht16")
        halo_b16 = spool.tile([64, XP], F16d, tag="hb16")
        lsum_p = spool.tile([128, 1], F32d, tag="lsump")
        lsum_a = spool.tile([128, 1], F32d, tag="lsuma")
        gs8 = spool.tile([8, 1], F32d, tag="gs8")
        gs8r = spool.tile([8, 1], F32d, tag="gs8r")
        done = spool.tile([1, 1], F32d, tag="done")
        iscv = spool.tile([1, 1], F32d, tag="iscv")
        notd = spool.tile([1, 1], F32d, tag="notd")
        steps = spool.tile([1, 1], F32d, tag="steps")
        stepsi = spool.tile([1, 1], mybir.dt.int32, tag="stepsi")
        doneb = spool.tile([128, 1], F32d, tag="doneb")
        gdt = spool.tile([128, 1], F32d, tag="gdt")

        nc.vector.memset(f16p[:], 0.0)
        nc.vector.memset(halo_t16[:], 0.0)
        nc.vector.memset(halo_b16[:], 0.0)
        nc.vector.memset(done[:], 0.0)
        nc.vector.memset(steps[:], 0.0)
        nc.vector.memset(notd[:], 1.0)
        nc.vector.memset(gdt[:], DT)
        nc.sync.dma_start(fa[:], f0_e[:])

        def fsrc(r0, nrows, dy, dx, plo=0, phi=128):
            """fp16 padded-field view: nrows rows starting at dst row r0,
            shifted by (dy, dx); [phi-plo, nrows, W]."""
            base = 2 + XP * (r0 + dy) + dx
            return f16p[plo:phi, base:base + XP * nrows] \
                .rearrange("p (y x) -> p y x", x=XP)[:, :, 0:W]

        PAIRS = [[2 * b, 2 * b + 1] for b in range(NCORES // 2)]

        def emit_payload(fn, with_lsum):
            agin = dram.tile([1, PAYLOAD], F32d, tag="agin")
            agout = dram.tile([2, PAYLOAD], F32d, tag="agout")
            top = agin[0:1, 0:C * W].rearrange("a (c x) -> (a c) x", c=C)
            bot = agin[0:1, C * W:2 * C * W].rearrange("a (c x) -> (a c) x", c=C)
            nc.sync.dma_start(top, fn[0:64, 0:W])
            nc.sync.dma_start(bot, fn[64:128, FREE - W:FREE])
            nc.gpsimd.collective_compute(
                "AllGather", mybir.AluOpType.bypass,
                replica_groups=PAIRS,
                ins=[agin[:].opt()], outs=[agout[:].opt()])
            if with_lsum:
                lin = dram.tile([1, 1], F32d, tag="lin")
                lout = dram.tile([8, 1], F32d, tag="lout", addr_space="Shared")
                nc.sync.dma_start(lin[:], lsum_a[0:1, 0:1])
                nc.gpsimd.collective_compute(
                    "AllGather", mybir.AluOpType.bypass,
                    replica_groups=[list(range(NCORES))],
                    ins=[lin[:].opt()], outs=[lout[:].opt()])
            else:
                lout = None
            return agout, lout

        def consume_collective(agpair, with_gate):
            agout, lout = agpair
            for j in (0, 1):
                nc.sync.dma_start(
                    cand[j][:],
                    agout[j:j + 1, C * W:2 * C * W]
                    .rearrange("a (c x) -> (a c) x", c=C))
                nc.sync.dma_start(
                    cand[2 + j][:],
                    agout[j:j + 1, 0:C * W]
                    .rearrange("a (c x) -> (a c) x", c=C))
            # masked partner select (per-core mask data)
            nc.vector.tensor_scalar(halo_t32[:], cand[0][:],
                                    hmask[:, 0:1], None, mybir.AluOpType.mult)
            nc.vector.scalar_tensor_tensor(
                halo_t32[:], cand[1][:], hmask[:, 1:2], halo_t32[:],
                mybir.AluOpType.mult, mybir.AluOpType.add)
            nc.vector.tensor_scalar(halo_b32[:], cand[2][:],
                                    hmask[:, 2:3], None, mybir.AluOpType.mult)
            nc.vector.scalar_tensor_tensor(
                halo_b32[:], cand[3][:], hmask[:, 3:4], halo_b32[:],
                mybir.AluOpType.mult, mybir.AluOpType.add)
            nc.vector.tensor_copy(halo_t16[:, 1:1 + W], halo_t32[:])
            nc.vector.tensor_copy(halo_b16[:, 1:1 + W], halo_b32[:])
            if with_gate:
                nc.sync.dma_start(gs8[:], lout[:, :])
                nc.gpsimd.partition_all_reduce(
                    gs8r[:], gs8[:], channels=8,
                    reduce_op=bass_isa.ReduceOp.add)
                nc.vector.tensor_scalar(
                    iscv[:], gs8r[0:1, 0:1], float(THRESH_SUM), None,
                    mybir.AluOpType.is_lt)
                nc.vector.tensor_tensor(done[:], done[:], iscv[:],
                                        mybir.AluOpType.max)
                nc.vector.tensor_scalar(notd[:], done[:], -1.0, 1.0,
                                        mybir.AluOpType.mult,
                                        mybir.AluOpType.add)
                nc.gpsimd.partition_broadcast(doneb[:], done[0:1, :])
                nc.vector.tensor_scalar(gdt[:], doneb[:], -DT, DT,
                                        mybir.AluOpType.mult,
                                        mybir.AluOpType.add)
            nc.vector.tensor_tensor(steps[:], steps[:], notd[:],
                                    mybir.AluOpType.add)

        agout_prev = emit_payload(fa, with_lsum=False)

        for k in range(nsteps):
            fc, fn = (fa, fb) if k % 2 == 0 else (fb, fa)
            consume_collective(agout_prev, with_gate=(k >= 1))

            # fp32 master -> fp16 padded (GpSimd), chunked
            for j in range(NCHUNK):
                r0 = j * RPC
                nc.gpsimd.tensor_copy(
                    fsrc(r0, RPC, 0, 0),
                    fc[:, r0 * W:(r0 + RPC) * W]
                    .rearrange("p (y x) -> p y x", x=W))

            # DVE chain: center tap (full rows, + b2 tile), then dy=+-1 dx=0
            for j in range(NCHUNK):
                r0, r1 = j * RPC, (j + 1) * RPC
                nc.vector.scalar_tensor_tensor(
                    acc[:, r0 * W:r1 * W], fsrc(r0, RPC, 0, 0),
                    stap[:, 4:5], b2t[:, r0 * W:r1 * W],
                    mybir.AluOpType.mult, mybir.AluOpType.add)
            for dy in (-1, 1):
                t = (dy + 1) * 3 + 1
                for j in range(NCHUNK):
                    a = max(j * RPC, 1) if dy == -1 else j * RPC
                    b = (j + 1) * RPC if dy == -1 else min((j + 1) * RPC, 31)
                    nc.vector.scalar_tensor_tensor(
                        acc[:, a * W:b * W], fsrc(a, b - a, dy, 0),
                        stap[:, t:t + 1], acc[:, a * W:b * W],
                        mybir.AluOpType.mult, mybir.AluOpType.add)

            # mm1 + gelu per (slab, chunk)
            for s in (0, 1):
                for j in range(NCHUNK):
                    ph = hpsum.tile([128, CH], F32d, tag="ph")
                    for m in (0, 1):
                        r0 = j * RPC + m * (RPC // 2)
                        nc.tensor.matmul(
                            ph[:, m * 512:(m + 1) * 512],
                            w1t[64 * s:64 * s + 64, :],
                            fsrc(r0, RPC // 2, 0, 0, 64 * s, 64 * s + 64),
                            start=True, stop=True,
                            tile_position=(64 * s, 0))
                    nc.scalar.activation(
                        hs[s][:, j * CH:(j + 1) * CH], ph[:],
                        mybir.ActivationFunctionType.Gelu,
                        bias=b1v[:], scale=1.0)

            # deriv PSUM accumulation per chunk: mm2 + PE taps + corrections
            for j in range(NCHUNK):
                pd = dpsum.tile([128, CH], F32d, tag="pd")
                mms = [[], []]   # per 512-bank: (args, kwargs) list
                tail = [[], []]  # trailing full-partition taps (stop target)
                for m in (0, 1):
                    nlo = j * CH + m * 512
                    for ti, (dy, dx) in enumerate(TAPS_PE):
                        r0 = j * RPC + m * (RPC // 2)
                        a = max(r0, -dy) if dy < 0 else r0
                        b = r0 + RPC // 2 if dy <= 0 \
                            else min(r0 + RPC // 2, SLAB - dy)
                        if a >= b:
                            continue
                        t = (dy + 1) * 3 + (dx + 1)
                        dst = [] if ti == 0 else tail
                        (mms if ti == 0 else tail)[m].append((
                            (pd[:, m * 512 + (a - r0) * W:
                                 m * 512 + (b - r0) * W],
                             dtap[:, t * 128:(t + 1) * 128],
                             fsrc(a, b - a, dy, dx)),
                            dict(tile_position=(0, 0))))
                    for s in (0, 1):
                        mms[m].append((
                            (pd[64 * s:64 * s + 64, m * 512:(m + 1) * 512],
                             w2t[:], hs[s][:, nlo:nlo + 512]),
                            dict(tile_position=(0, 64 * s))))
                # boundary-row corrections (all dy=+-1 taps incl. dx==0)
                if j == 0:
                    for i, dx in enumerate((-1, 0, 1)):
                        # cross-slab dy=-1: slab0 row31 -> slab1 row0
                        mms[0].append((
                            (pd[64:128, 0:W], d64[0:64, i * 64:(i + 1) * 64],
                             fsrc(SLAB - 1, 1, 0, dx, 0, 64)[:, 0, :]),
                            dict(tile_position=(0, 64))))
                        # halo-top: above-halo row -> slab0 row0
                        mms[0].append((
                            (pd[0:64, 0:W], dhalo[:, i * 64:(i + 1) * 64],
                             halo_t16[:, 1 + dx:1 + dx + W]),
                            dict(tile_position=(0, 0))))
                if j == NCHUNK - 1:
                    off = CH - W   # last row of the chunk
                    for i, dx in enumerate((-1, 0, 1)):
                        # cross-slab dy=+1: slab1 row0 -> slab0 row31
                        mms[1].append((
                            (pd[0:64, off:CH],
                             d64[64:128, i * 64:(i + 1) * 64],
                             fsrc(0, 1, 0, dx, 64, 128)[:, 0, :]),
                            dict(tile_position=(64, 0))))
                        # halo-bottom: below-halo row -> slab1 row31
                        mms[1].append((
                            (pd[64:128, off:CH],
                             dhalo[:, (3 + i) * 64:(4 + i) * 64],
                             halo_b16[:, 1 + dx:1 + dx + W]),
                            dict(tile_position=(0, 64))))
                for m in (0, 1):
                    seq = mms[m] + tail[m]
                    n = len(seq)
                    for i, (args, kw) in enumerate(seq):
                        nc.tensor.matmul(*args, start=(i == 0),
                                         stop=(i == n - 1), **kw)
                # merge: der = (psum * 1) + acc   (fp16 out)
                nc.vector.scalar_tensor_tensor(
                    der[:, j * CH:(j + 1) * CH], pd[:], 1.0,
                    acc[:, j * CH:(j + 1) * CH],
                    mybir.AluOpType.mult, mybir.AluOpType.add)

            # |deriv| sum (ACT abs + accum), cross-partition reduce
            nc.scalar.activation(junk[:], der[:],
                                 mybir.ActivationFunctionType.Abs,
                                 accum_out=lsum_p[:])
            nc.gpsimd.partition_all_reduce(
                lsum_a[:], lsum_p[:], channels=128,
                reduce_op=bass_isa.ReduceOp.add)

            # master update (DVE), chunked
            for j in range(NCHUNK):
                nc.vector.scalar_tensor_tensor(
                    fn[:, j * CH:(j + 1) * CH], der[:, j * CH:(j + 1) * CH],
                    gdt[:], fc[:, j * CH:(j + 1) * CH],
                    mybir.AluOpType.mult, mybir.AluOpType.add)

            if k < nsteps - 1:
                agout_prev = emit_payload(fn, with_lsum=True)

        ffinal = fa if nsteps % 2 == 0 else fb
        nc.sync.dma_start(outf_e[:], ffinal[:])
        nc.vector.tensor_copy(stepsi[:], steps[:])
        nc.sync.dma_start(steps_e[:], stepsi[:])

    nc.compile()
    return nc


def _get_nc(nsteps):
    if nsteps not in _NC_CACHE:
        _NC_CACHE[nsteps] = build_nc(nsteps)
    return _NC_CACHE[nsteps]


def _prep_inputs(field, dw, db, w1, b1, w2, b2, dcoeff):
    """Host-side sharding + derived parameter tensors."""
    field = np.ascontiguousarray(field, np.float32)
    s = (np.float32(dcoeff.reshape(-1)[0]) * dw[:, 0]).astype(np.float32)  # [64,3,3]
    b2eff = (b2 + np.float32(dcoeff.reshape(-1)[0]) * db).astype(np.float32)

    w1t = np.vstack([w1.T, w1.T]).astype(np.float16)           # [128,128]
    w2t = w2.T.astype(np.float16)                              # [128,64]
    b1v = b1.reshape(128, 1).astype(np.float32)
    b2t = np.broadcast_to(
        np.concatenate([b2eff, b2eff]).reshape(128, 1), (128, FREE)) \
        .astype(np.float16)
    stapm = np.zeros((128, 9), np.float32)
    for dy in range(3):
        for dx in range(3):
            t = dy * 3 + dx
            stapm[0:64, t] = s[:, dy, dx]
            stapm[64:128, t] = s[:, dy, dx]
    dtap = np.zeros((9, 128, 128), np.float16)
    for t in range(9):
        np.fill_diagonal(dtap[t], stapm[:, t].astype(np.float16))
    dtap = dtap.transpose(1, 0, 2).reshape(128, 9 * 128)
    d64 = np.zeros((128, 3, 64), np.float16)
    for i in range(3):
        np.fill_diagonal(d64[0:64, i], s[:, 0, i].astype(np.float16))
        np.fill_diagonal(d64[64:128, i], s[:, 2, i].astype(np.float16))
    d64 = d64.reshape(128, 192)

    in_maps = []
    for core in range(NCORES):
        b, h = core // 2, core % 2
        sub = field[b, :, 64 * h:64 * h + 64, :]         # [64, 64, 128]
        f0 = sub.reshape(C, 2, SLAB, W).transpose(1, 0, 2, 3) \
            .reshape(128, FREE).astype(np.float32)
        hm = np.zeros((64, 4), np.float32)
        hm[:, 1 - h] = 1.0       # top candidate: partner slot in the pair
        hm[:, 2 + (1 - h)] = 1.0  # bottom candidate
        dhalo = np.zeros((64, 6, 64), np.float16)
        for i in range(3):
            if h == 1:   # real row above exists
                np.fill_diagonal(dhalo[:, i], s[:, 0, i].astype(np.float16))
            if h == 0:   # real row below exists
                np.fill_diagonal(dhalo[:, 3 + i], s[:, 2, i].astype(np.float16))
        in_maps.append({
            "f0": np.ascontiguousarray(f0),
            "w1t": w1t, "w2t": w2t, "b1v": b1v,
            "b2t": np.ascontiguousarray(b2t), "stap": stapm,
            "dtap": np.ascontiguousarray(dtap),
            "d64": np.ascontiguousarray(d64),
            "dhalo": np.ascontiguousarray(dhalo.reshape(64, 6 * 64)),
            "hmask": np.ascontiguousarray(hm),
        })
    return in_maps


def _unshard(results):
    out = np.zeros((B, C, H, W), np.float32)
    for core in range(NCORES):
        b, h = core // 2, core % 2
        t = results[core]["outf"].reshape(2, C, SLAB, W).transpose(1, 0, 2, 3)
        out[b, :, 64 * h:64 * h + 64, :] = t.reshape(C, 64, W)
    steps = int(results[0]["steps"].reshape(-1)[0])
    return out, steps


def kernel(field, dw, db, w1, b1, w2, b2, dcoeff, max_steps, _trace=False):
    nsteps = int(max_steps)
    in_maps = _prep_inputs(np.asarray(field), np.asarray(dw, np.float32),
                           np.asarray(db, np.float32),
                           np.asarray(w1, np.float32),
                           np.asarray(b1, np.float32),
                           np.asarray(w2, np.float32),
                           np.asarray(b2, np.float32),
                           np.asarray(dcoeff, np.float32))
    nc = _get_nc(nsteps)
    res = run_bass_kernel_spmd(nc, in_maps, core_ids=list(range(NCORES)),
                               trace=_trace)
    out, steps = _unshard(res.results)
    if _trace:
        kernel.last_exec_ns = res.exec_time_ns
        kernel.last_trace = res.instructions_and_trace
    return out, np.int32(steps)


# revision 15
# speedup vs baseline: 1.0797x; 1.0797x over previous
"""AdaptiveFieldEvolution Trainium2 kernel (8 NeuronCores, SPMD).

Semantics (matches the fp32 reference): 50 Euler steps of
  f += DT*(dcoeff*depthwise3x3(f) + db + W2@gelu(W1@f + b1) + b2)
with a global freeze once mean|df| < THRESH, and an adaptive `steps` output.

Sharding: batch(4) x H-halves(2) -> 8 cores. Per-core tile [128, 4096]:
partition = slab(2)*64 + channel(64), free = y(32) x W(128); each slab is a
32-row band. fp32 master field, fp16 compute, fp32 PSUM accumulation.

Per step: PE runs the two 1x1-conv matmuls plus the six dx!=0 depthwise taps
(diagonal lhsT, PSUM-accumulated) and all boundary-row corrections; ACT runs
gelu (+b1, fp16 out) and the |deriv| abs-sum; DVE runs the three dx==0 taps
as an fp16 chain (+b2 tile) plus the PSUM merge and the fp32 master update;
GpSimd does the fp32->fp16 padded cast and the convergence bookkeeping.
One AllGather per step carries both halo edge rows + the local |deriv| sum;
per-core *data* (zeroed halo-tap weights at the global boundary, a partner
row index consumed via a register-offset DMA) keeps the SPMD graph identical
on all cores.
"""
import sys
import types
import numpy as np

sys.path.insert(0, "/opt/trn_rl_repo")


def _install_prof_shim():
    try:
        from antenv import axon_hooks  # noqa: F401
        return
    except ImportError:
        pass
    import antenv
    mod = types.ModuleType("antenv.axon_hooks")
    _state = {"hook": None}
    mod.set_axon_ntff_profile_hook = lambda h: _state.__setitem__("hook", h)
    mod.get_axon_ntff_profile_hook = lambda: _state["hook"]
    sys.modules["antenv.axon_hooks"] = mod
    antenv.axon_hooks = mod
    try:
        from trn_agent_boot.trn_boot import _ntff_profile_via_ctypes
        mod.set_axon_ntff_profile_hook(
            _ntff_profile_via_ctypes("/opt/axon/libaxon_pjrt.so"))
    except Exception:
        pass


_install_prof_shim()

import concourse.bass as bass  # noqa: E402
import concourse.tile as tile  # noqa: E402
from concourse import bacc, mybir, bass_isa  # noqa: E402
from concourse.bass_utils import run_bass_kernel_spmd  # noqa: E402
from contextlib import ExitStack  # noqa: E402

B, C, H, W = 4, 64, 128, 128
NCORES = 8
DT = 0.1
THRESH = 0.01
SLAB = 32                      # rows per slab half
FREE = SLAB * W                # 4096 real free elems per partition
XP = W + 2                     # padded row stride
FPAD = 2 + SLAB * XP + 2       # padded fp16 free size, front+tail pads
PAYLOAD = 2 * C * W            # top-row block + bottom-row block
NCHUNK = 4
CH = FREE // NCHUNK            # 1024 (one PSUM tile = 2 banks)
RPC = SLAB // NCHUNK           # 8 rows per chunk
F32d = mybir.dt.float32
F16d = mybir.dt.float16
THRESH_SUM = THRESH * (B * C * H * W) / DT

_NC_CACHE = {}


def build_nc(nsteps):
    nc = bacc.Bacc("TRN2", target_bir_lowering=False, debug=False,
                   num_devices=NCORES)
    f0_e = nc.dram_tensor("f0", [128, FREE], F32d, kind="ExternalInput")
    w1t_e = nc.dram_tensor("w1t", [128, 128], F16d, kind="ExternalInput")
    w2t_e = nc.dram_tensor("w2t", [128, 64], F16d, kind="ExternalInput")
    b1v_e = nc.dram_tensor("b1v", [128, 1], F32d, kind="ExternalInput")
    b2p_e = nc.dram_tensor("b2p", [128, 1], F32d, kind="ExternalInput")
    stap_e = nc.dram_tensor("stap", [128, 9], F32d, kind="ExternalInput")
    dtap_e = nc.dram_tensor("dtap", [128, 9 * 128], F16d, kind="ExternalInput")
    d64_e = nc.dram_tensor("d64", [128, 3 * 64], F16d, kind="ExternalInput")
    dhalo_e = nc.dram_tensor("dhalo", [64, 6 * 64], F16d, kind="ExternalInput")
    hmask_e = nc.dram_tensor("hmask", [64, 4], F32d, kind="ExternalInput")
    outf_e = nc.dram_tensor("outf", [128, FREE], F32d, kind="ExternalOutput")
    steps_e = nc.dram_tensor("steps", [1, 1], mybir.dt.int32, kind="ExternalOutput")

    # dy=0 taps first: the per-bank opener must cover every row/col/partition
    TAPS_PE = [(0, 0), (0, -1), (0, 1), (-1, -1), (-1, 0), (-1, 1),
               (1, -1), (1, 0), (1, 1)]

    with tile.TileContext(nc) as tc, ExitStack() as ctx:
        cpool = ctx.enter_context(tc.tile_pool(name="consts", bufs=1))
        spool = ctx.enter_context(tc.tile_pool(name="state", bufs=1))
        hpsum = ctx.enter_context(tc.tile_pool(name="hpsum", bufs=2, space="PSUM"))
        dpsum = ctx.enter_context(tc.tile_pool(name="dpsum", bufs=2, space="PSUM"))
        dram = ctx.enter_context(tc.tile_pool(name="dram", bufs=2, space="DRAM"))

        # ---- constants ----
        w1t = cpool.tile([128, 128], F16d, tag="w1t")
        w2t = cpool.tile([128, 64], F16d, tag="w2t")
        b1v = cpool.tile([128, 1], F32d, tag="b1v")
        b2p = cpool.tile([128, 1], F32d, tag="b2p")
        stap = cpool.tile([128, 9], F32d, tag="stap")
        dtap = cpool.tile([128, 9 * 128], F16d, tag="dtap")
        d64 = cpool.tile([128, 3 * 64], F16d, tag="d64")
        dhalo = cpool.tile([64, 6 * 64], F16d, tag="dhalo")
        hmask = cpool.tile([64, 4], F32d, tag="hmask")
        for t, e in ((w1t, w1t_e), (w2t, w2t_e), (b1v, b1v_e), (b2p, b2p_e),
                     (stap, stap_e), (dtap, dtap_e), (d64, d64_e),
                     (dhalo, dhalo_e), (hmask, hmask_e)):
            nc.sync.dma_start(t[:], e[:])

        # ---- state ----
        fa = spool.tile([128, FREE], F32d, tag="fa")
        fb = spool.tile([128, FREE], F32d, tag="fb")
        f16p = spool.tile([128, FPAD], F16d, tag="f16p")
        h0 = spool.tile([128, FREE], F16d, tag="h0")
        h1 = spool.tile([128, FREE], F16d, tag="h1")
        hs = (h0, h1)
        der = spool.tile([128, FREE], F16d, tag="der")
        junk = spool.tile([128, FREE], F16d, tag="junk")
        halo_t32 = spool.tile([64, W], F32d, tag="ht32")
        halo_b32 = spool.tile([64, W], F32d, tag="hb32")
        cand = [spool.tile([64, W], F32d, tag=f"cand{i}", name=f"cand{i}")
                for i in range(4)]
        halo_t16 = spool.tile([64, XP], F16d, tag="ht16")
        halo_b16 = spool.tile([64, XP], F16d, tag="hb16")
        lsum_p = spool.tile([128, 1], F32d, tag="lsump")
        lsum_a = spool.tile([128, 1], F32d, tag="lsuma")
        gs8 = spool.tile([8, 1], F32d, tag="gs8")
        gs8r = spool.tile([8, 1], F32d, tag="gs8r")
        done = spool.tile([1, 1], F32d, tag="done")
        iscv = spool.tile([1, 1], F32d, tag="iscv")
        notd = spool.tile([1, 1], F32d, tag="notd")
        steps = spool.tile([1, 1], F32d, tag="steps")
        stepsi = spool.tile([1, 1], mybir.dt.int32, tag="stepsi")
        doneb = spool.tile([128, 1], F32d, tag="doneb")
        gdt = spool.tile([128, 1], F32d, tag="gdt")

        nc.vector.memset(f16p[:], 0.0)
        nc.vector.memset(halo_t16[:], 0.0)
        nc.vector.memset(halo_b16[:], 0.0)
        nc.vector.memset(done[:], 0.0)
        nc.vector.memset(steps[:], 0.0)
        nc.vector.memset(notd[:], 1.0)
        nc.vector.memset(gdt[:], DT)
        nc.sync.dma_start(fa[:], f0_e[:])

        def fsrc(r0, nrows, dy, dx, plo=0, phi=128):
            """fp16 padded-field view: nrows rows starting at dst row r0,
            shifted by (dy, dx); [phi-plo, nrows, W]."""
            base = 2 + XP * (r0 + dy) + dx
            return f16p[plo:phi, base:base + XP * nrows] \
                .rearrange("p (y x) -> p y x", x=XP)[:, :, 0:W]

        PAIRS = [[2 * b, 2 * b + 1] for b in range(NCORES // 2)]

        def emit_payload(fn, with_lsum):
            agin = dram.tile([1, PAYLOAD], F32d, tag="agin")
            agout = dram.tile([2, PAYLOAD], F32d, tag="agout")
            top = agin[0:1, 0:C * W].rearrange("a (c x) -> (a c) x", c=C)
            bot = agin[0:1, C * W:2 * C * W].rearrange("a (c x) -> (a c) x", c=C)
            nc.sync.dma_start(top, fn[0:64, 0:W])
            nc.sync.dma_start(bot, fn[64:128, FREE - W:FREE])
            nc.gpsimd.collective_compute(
                "AllGather", mybir.AluOpType.bypass,
                replica_groups=PAIRS,
                ins=[agin[:].opt()], outs=[agout[:].opt()])
            if with_lsum:
                lin = dram.tile([1, 1], F32d, tag="lin")
                lout = dram.tile([8, 1], F32d, tag="lout", addr_space="Shared")
                nc.sync.dma_start(lin[:], lsum_a[0:1, 0:1])
                nc.gpsimd.collective_compute(
                    "AllGather", mybir.AluOpType.bypass,
                    replica_groups=[list(range(NCORES))],
                    ins=[lin[:].opt()], outs=[lout[:].opt()])
            else:
                lout = None
            return agout, lout

        def consume_collective(agpair, with_gate):
            agout, lout = agpair
            for j in (0, 1):
                nc.sync.dma_start(
                    cand[j][:],
                    agout[j:j + 1, C * W:2 * C * W]
                    .rearrange("a (c x) -> (a c) x", c=C))
                nc.sync.dma_start(
                    cand[2 + j][:],
                    agout[j:j + 1, 0:C * W]
                    .rearrange("a (c x) -> (a c) x", c=C))
            # masked partner select (per-core mask data)
            nc.vector.tensor_scalar(halo_t32[:], cand[0][:],
                                    hmask[:, 0:1], None, mybir.AluOpType.mult)
            nc.vector.scalar_tensor_tensor(
                halo_t32[:], cand[1][:], hmask[:, 1:2], halo_t32[:],
                mybir.AluOpType.mult, mybir.AluOpType.add)
            nc.vector.tensor_scalar(halo_b32[:], cand[2][:],
                                    hmask[:, 2:3], None, mybir.AluOpType.mult)
            nc.vector.scalar_tensor_tensor(
                halo_b32[:], cand[3][:], hmask[:, 3:4], halo_b32[:],
                mybir.AluOpType.mult, mybir.AluOpType.add)
            nc.vector.tensor_copy(halo_t16[:, 1:1 + W], halo_t32[:])
            nc.vector.tensor_copy(halo_b16[:, 1:1 + W], halo_b32[:])
            if with_gate:
                nc.sync.dma_start(gs8[:], lout[:, :])
                nc.gpsimd.partition_all_reduce(
                    gs8r[:], gs8[:], channels=8,
                    reduce_op=bass_isa.ReduceOp.add)
                nc.vector.tensor_scalar(
                    iscv[:], gs8r[0:1, 0:1], float(THRESH_SUM), None,
                    mybir.AluOpType.is_lt)
                nc.vector.tensor_tensor(done[:], done[:], iscv[:],
                                        mybir.AluOpType.max)
                nc.vector.tensor_scalar(notd[:], done[:], -1.0, 1.0,
                                        mybir.AluOpType.mult,
                                        mybir.AluOpType.add)
                nc.gpsimd.partition_broadcast(doneb[:], done[0:1, :])
                nc.vector.tensor_scalar(gdt[:], doneb[:], -DT, DT,
                                        mybir.AluOpType.mult,
                                        mybir.AluOpType.add)
            nc.vector.tensor_tensor(steps[:], steps[:], notd[:],
                                    mybir.AluOpType.add)

        agout_prev = emit_payload(fa, with_lsum=False)

        for k in range(nsteps):
            fc, fn = (fa, fb) if k % 2 == 0 else (fb, fa)
            consume_collective(agout_prev, with_gate=(k >= 1))

            # fp32 master -> fp16 padded (GpSimd), chunked
            for j in range(NCHUNK):
                r0 = j * RPC
                nc.vector.tensor_copy(
                    fsrc(r0, RPC, 0, 0),
                    fc[:, r0 * W:(r0 + RPC) * W]
                    .rearrange("p (y x) -> p y x", x=W))

            # mm1 + gelu per (slab, chunk)
            for s in (0, 1):
                for j in range(NCHUNK):
                    ph = hpsum.tile([128, CH], F32d, tag="ph")
                    for m in (0, 1):
                        r0 = j * RPC + m * (RPC // 2)
                        nc.tensor.matmul(
                            ph[:, m * 512:(m + 1) * 512],
                            w1t[64 * s:64 * s + 64, :],
                            fsrc(r0, RPC // 2, 0, 0, 64 * s, 64 * s + 64),
                            start=True, stop=True,
                            tile_position=(64 * s, 0))
                    nc.scalar.activation(
                        hs[s][:, j * CH:(j + 1) * CH], ph[:],
                        mybir.ActivationFunctionType.Gelu,
                        bias=b1v[:], scale=1.0)

            # deriv PSUM accumulation per chunk: mm2 + PE taps + corrections
            for j in range(NCHUNK):
                pd = dpsum.tile([128, CH], F32d, tag="pd")
                mms = [[], []]   # per 512-bank: (args, kwargs) list
                tail = [[], []]  # trailing full-partition taps (stop target)
                for m in (0, 1):
                    nlo = j * CH + m * 512
                    for ti, (dy, dx) in enumerate(TAPS_PE):
                        r0 = j * RPC + m * (RPC // 2)
                        a = max(r0, -dy) if dy < 0 else r0
                        b = r0 + RPC // 2 if dy <= 0 \
                            else min(r0 + RPC // 2, SLAB - dy)
                        if a >= b:
                            continue
                        t = (dy + 1) * 3 + (dx + 1)
                        dst = [] if ti == 0 else tail
                        (mms if ti == 0 else tail)[m].append((
                            (pd[:, m * 512 + (a - r0) * W:
                                 m * 512 + (b - r0) * W],
                             dtap[:, t * 128:(t + 1) * 128],
                             fsrc(a, b - a, dy, dx)),
                            dict(tile_position=(0, 0))))
                    for s in (0, 1):
                        mms[m].append((
                            (pd[64 * s:64 * s + 64, m * 512:(m + 1) * 512],
                             w2t[:], hs[s][:, nlo:nlo + 512]),
                            dict(tile_position=(0, 64 * s))))
                # boundary-row corrections (all dy=+-1 taps incl. dx==0)
                if j == 0:
                    for i, dx in enumerate((-1, 0, 1)):
                        # cross-slab dy=-1: slab0 row31 -> slab1 row0
                        mms[0].append((
                            (pd[64:128, 0:W], d64[0:64, i * 64:(i + 1) * 64],
                             fsrc(SLAB - 1, 1, 0, dx, 0, 64)[:, 0, :]),
                            dict(tile_position=(0, 64))))
                        # halo-top: above-halo row -> slab0 row0
                        mms[0].append((
                            (pd[0:64, 0:W], dhalo[:, i * 64:(i + 1) * 64],
                             halo_t16[:, 1 + dx:1 + dx + W]),
                            dict(tile_position=(0, 0))))
                if j == NCHUNK - 1:
                    off = CH - W   # last row of the chunk
                    for i, dx in enumerate((-1, 0, 1)):
                        # cross-slab dy=+1: slab1 row0 -> slab0 row31
                        mms[1].append((
                            (pd[0:64, off:CH],
                             d64[64:128, i * 64:(i + 1) * 64],
                             fsrc(0, 1, 0, dx, 64, 128)[:, 0, :]),
                            dict(tile_position=(64, 0))))
                        # halo-bottom: below-halo row -> slab1 row31
                        mms[1].append((
                            (pd[64:128, off:CH],
                             dhalo[:, (3 + i) * 64:(4 + i) * 64],
                             halo_b16[:, 1 + dx:1 + dx + W]),
                            dict(tile_position=(0, 64))))
                for m in (0, 1):
                    seq = mms[m] + tail[m]
                    n = len(seq)
                    for i, (args, kw) in enumerate(seq):
                        nc.tensor.matmul(*args, start=(i == 0),
                                         stop=(i == n - 1), **kw)
                # merge/drain: der = psum + b2 (fp16 out)
                nc.vector.tensor_scalar(
                    der[:, j * CH:(j + 1) * CH], pd[:], b2p[:, 0:1], None,
                    mybir.AluOpType.add)

            # |deriv| sum (ACT abs + accum), cross-partition reduce
            nc.scalar.activation(junk[:], der[:],
                                 mybir.ActivationFunctionType.Abs,
                                 accum_out=lsum_p[:])
            nc.gpsimd.partition_all_reduce(
                lsum_a[:], lsum_p[:], channels=128,
                reduce_op=bass_isa.ReduceOp.add)

            # master update (DVE), chunked
            for j in range(NCHUNK):
                nc.vector.scalar_tensor_tensor(
                    fn[:, j * CH:(j + 1) * CH], der[:, j * CH:(j + 1) * CH],
                    gdt[:], fc[:, j * CH:(j + 1) * CH],
                    mybir.AluOpType.mult, mybir.AluOpType.add)

            if k < nsteps - 1:
                agout_prev = emit_payload(fn, with_lsum=True)

        ffinal = fa if nsteps % 2 == 0 else fb
        nc.sync.dma_start(outf_e[:], ffinal[:])
        nc.vector.tensor_copy(stepsi[:], steps[:])
        nc.sync.dma_start(steps_e[:], stepsi[:])

    nc.compile()
    return nc


def _get_nc(nsteps):
    if nsteps not in _NC_CACHE:
        _NC_CACHE[nsteps] = build_nc(nsteps)
    return _NC_CACHE[nsteps]


def _prep_inputs(field, dw, db, w1, b1, w2, b2, dcoeff):
    """Host-side sharding + derived parameter tensors."""
    field = np.ascontiguousarray(field, np.float32)
    s = (np.float32(dcoeff.reshape(-1)[0]) * dw[:, 0]).astype(np.float32)  # [64,3,3]
    b2eff = (b2 + np.float32(dcoeff.reshape(-1)[0]) * db).astype(np.float32)

    w1t = np.vstack([w1.T, w1.T]).astype(np.float16)           # [128,128]
    w2t = w2.T.astype(np.float16)                              # [128,64]
    b1v = b1.reshape(128, 1).astype(np.float32)
    b2p = np.concatenate([b2eff, b2eff]).reshape(128, 1).astype(np.float32)
    stapm = np.zeros((128, 9), np.float32)
    for dy in range(3):
        for dx in range(3):
            t = dy * 3 + dx
            stapm[0:64, t] = s[:, dy, dx]
            stapm[64:128, t] = s[:, dy, dx]
    dtap = np.zeros((9, 128, 128), np.float16)
    for t in range(9):
        np.fill_diagonal(dtap[t], stapm[:, t].astype(np.float16))
    dtap = dtap.transpose(1, 0, 2).reshape(128, 9 * 128)
    d64 = np.zeros((128, 3, 64), np.float16)
    for i in range(3):
        np.fill_diagonal(d64[0:64, i], s[:, 0, i].astype(np.float16))
        np.fill_diagonal(d64[64:128, i], s[:, 2, i].astype(np.float16))
    d64 = d64.reshape(128, 192)

    in_maps = []
    for core in range(NCORES):
        b, h = core // 2, core % 2
        sub = field[b, :, 64 * h:64 * h + 64, :]         # [64, 64, 128]
        f0 = sub.reshape(C, 2, SLAB, W).transpose(1, 0, 2, 3) \
            .reshape(128, FREE).astype(np.float32)
        hm = np.zeros((64, 4), np.float32)
        hm[:, 1 - h] = 1.0       # top candidate: partner slot in the pair
        hm[:, 2 + (1 - h)] = 1.0  # bottom candidate
        dhalo = np.zeros((64, 6, 64), np.float16)
        for i in range(3):
            if h == 1:   # real row above exists
                np.fill_diagonal(dhalo[:, i], s[:, 0, i].astype(np.float16))
            if h == 0:   # real row below exists
                np.fill_diagonal(dhalo[:, 3 + i], s[:, 2, i].astype(np.float16))
        in_maps.append({
            "f0": np.ascontiguousarray(f0),
            "w1t": w1t, "w2t": w2t, "b1v": b1v,
            "b2p": b2p, "stap": stapm,
            "dtap": np.ascontiguousarray(dtap),
            "d64": np.ascontiguousarray(d64),
            "dhalo": np.ascontiguousarray(dhalo.reshape(64, 6 * 64)),
            "hmask": np.ascontiguousarray(hm),
        })
    return in_maps


def _unshard(results):
    out = np.zeros((B, C, H, W), np.float32)
    for core in range(NCORES):
        b, h = core // 2, core % 2
        t = results[core]["outf"].reshape(2, C, SLAB, W).transpose(1, 0, 2, 3)
        out[b, :, 64 * h:64 * h + 64, :] = t.reshape(C, 64, W)
    steps = int(results[0]["steps"].reshape(-1)[0])
    return out, steps


def kernel(field, dw, db, w1, b1, w2, b2, dcoeff, max_steps, _trace=False):
    nsteps = int(max_steps)
    in_maps = _prep_inputs(np.asarray(field), np.asarray(dw, np.float32),
                           np.asarray(db, np.float32),
                           np.asarray(w1, np.float32),
                           np.asarray(b1, np.float32),
                           np.asarray(w2, np.float32),
                           np.asarray(b2, np.float32),
                           np.asarray(dcoeff, np.float32))
    nc = _get_nc(nsteps)
    res = run_bass_kernel_spmd(nc, in_maps, core_ids=list(range(NCORES)),
                               trace=_trace)
    out, steps = _unshard(res.results)
    if _trace:
        kernel.last_exec_ns = res.exec_time_ns
        kernel.last_trace = res.instructions_and_trace
    return out, np.int32(steps)
